# revision 27
# baseline (speedup 1.0000x reference)
"""Bahdanau additive attention scores on 8 TRN2 NeuronCores.

reference:
    h = hidden[-1]                                   # [B, He]
    e_proj = enc @ W_e;  h_proj = h @ W_h            # W_attn = [W_h; W_e]
    scores = tanh(h_proj[:,None,:] + e_proj + b) @ v # [B, S]
    out = softmax(scores, axis=1)

Strategy: pure data-parallel over batch (B=32 -> 4 per core), zero
collectives.  The 275 GFLOP GEMM is PE-bound, so the big levers are (1)
run EVERY hd-tile as float8e4 DoubleRow (K=256/instr, 2 mults/cycle --
the HW fp8 ceiling) and (2) cancel the resulting quantization error with
a first-order correction that costs the device nothing:

  CORR ("vmix0p" + corr): score err ~= sum_h v_h tanh'(x_h) eps_h with
  eps = enc@W - ehat@What.  Replace tanh'(x_h) by its mean
  m_h = E[sech^2(c + P)] (Gauss-Hermite over P~N(0,||W_col||^2), c known
  on host), giving two pieces:
    - ehat @ (dW u), u = m*v: linear in the STREAMED fp8 operand -> folded
      into the top-8 |v| weight columns before quantization (the h-columns
      are sorted by |v| desc; perturbing column h* by alpha/(v m) of the
      fold vector injects the correction through the tanh linearization).
      Zero device cost.
    - de @ (W u_b), de = enc - ehat, per-batch m_b: host GEMV -> [B,S]
      f32 map, DMA'd per pair and added to the scores row by one DVE
      [1,1024] op that doubles as the mandatory PSUM->SBUF move.
  rel err 8.928e-3 measured == sim_err.py prediction (vs 1.78e-2 plain
  all-fp8, 1.27e-2 for the old 1-bf16-tile vmix1p), gate 2e-2.

Device program per core (_emit_body_vmix_sp4, WSHARE=4): superpair = two
512-col-PAIR s-ranges processed group-locked.  Per hd-group g:
  4 fp8-DR matmul chains (pairA/B x half0/1, K=1024 as 4 k2-steps) into
      two [128,1024] 2-bank PSUM tiles; the 4 matmuls of each k2 step
      share one weight tile back-to-back (LDWEIGHTS reuse, measured
      -10.5us vs unshared)
  one pair-wide tanh per pair: ACT reads the 2-bank PSUM tile in ONE
      [128,1024] op (ACT costs (N+352)cyc/1.2GHz; halving the op count
      vs per-half tanh removed ACT co-criticality with PE)
  one pair-wide DVE scalar_tensor_tensor accumulates v_g*th into a f16
      acc (g-major order keeps the chain trailing each group's tanh, so
      the end-of-stream drain is one link, not the whole chain)
Scores finalization per pair is DEFERRED into the next superpair (g==2/
g==5 points): ones.T@accr halves -> [1,1024] scp PSUM, + corr (DVE),
Exp with per-pair accum_out; per batch: tot/reciprocal on DVE, 4 rescale
chunks DVE-only (an ACT burst would stall the 3-deep ep ring), each
chunk's out DMA overlapping the next.

Softmax skips the max-subtraction: scores are ~N(0, 0.65), |max| < ~4
over 128K samples, exp() is comfortably within f32 range.

Measured (HW, interleaved slope timing; fleet load moves absolutes +-25%,
NEVER compare across runs): same-run ladder vmix1p 367.4us -> this
kernel 308.4us (0.84x); under the prior session's load that graded
vmix1p at 288.5us this extrapolates to ~245us.  Attribution (interleaved
n65 deltas): PE matmul stream alone ~= full-33.6us with DMA fully hidden
(nodma == noact); tanh+DVE exposure ~4us; finish ~30us.  The PE stream
sits at the real DR floor (DR streams ~1col/cycle with LDW partially
serialized -- the 0.5cyc/row cost-model number is not achievable;
LDWEIGHTS reuse via WSHARE=4 recovers ~8us of it).

Rejected-but-measured (3 sessions):
  - f32r all tiles: 523us, 9.6e-5 (safe fallback: MM_DTYPE/mode knobs)
  - vmix1p (1 bf16 tile): 367.4us same-run vs 308.4 here
  - int8/uint8 matmul: walrus hard-rejects (s3d3_mm_dtype); fp8e3 DR:
    codegen rejects; DoubleColumn/DoublePixel: uint8-only -> all dead
  - FD=1024 matmul (2-bank PSUM out): walrus ISA check I-97 rejects;
    2-bank ACT READS are fine (this kernel relies on them)
  - HOST_SOFTMAX (device ships raw scores): no win (312.6 vs 308.4
    interleaved) -- Exp/rescale were already hidden; kept off
  - scp ring 2 paid by ep ring 2 (scp2/epp2): 332.0 vs 315.6 -- ep ring
    depth wins
  - per-batch fold weights (m_bh in the fold): only 8.65e-3 vs 8.93e-3
    in sim, not worth 4x weight SBUF/DMA

build_nc(n_loop=N) wraps the body in an in-NEFF For_i loop -- used by
test.py to amortize the ~80ms axon-tunnel dispatch cost when timing.
The graded path is build_nc() defaults (DEFAULT_MODE="vmix0p" + CORR +
PAIR2 + WSHARE=4).
"""

import numpy as np

import concourse.mybir as mybir
import concourse.tile as tile
from concourse import bacc
from concourse.bass_utils import run_bass_kernel_spmd

N_CORES = 8
L, B, S, He, Hd = 2, 32, 4096, 1024, 1024
BPC = B // N_CORES  # batches per core
KT = He // 128      # contraction tiles
HT = Hd // 128      # hd tiles
SB = 512            # s-block (matmul moving free dim)
NSB = S // SB
F32 = mybir.dt.float32
F32R = mybir.dt.float32r
BF16 = mybir.dt.bfloat16
F16 = mybir.dt.float16

# matmul-input dtype for the big GEMM:
#   "f32r"  - tf32-like, 1 col/cycle
#   "bf16"  - 1 col/cycle
#   "fp8dr" - float8e4 with DoubleRow: K=256 per instruction, 2 multiplies/cycle
MM_DTYPE = "f32r"

# default mode for the graded path.  "vmix<K>": h-columns sorted by |v_w|
# descending; top K hd-tiles computed in bf16 (1 col/cycle), bottom 8-K in
# scaled fp8e4 DoubleRow (2 mults/cycle).  Sorting concentrates ~73% of the
# score-error sensitivity (which scales with v_h^2) into the top 2 tiles,
# so KF=2 cuts rel err to 9.4e-3 while 6 of 8 tiles run at DR speed.
DEFAULT_MODE = "vmix0p"
# CORR: first-order cancellation of the fp8 quantization error, enabling
# the all-fp8 path (kf=0, 32 matmuls/block vs vmix1's 36).  The score error
# is ~ sum_h v_h tanh'(x_h) eps_h with eps = enc@W - ehat@What.  Replace
# tanh'(x_h) by its host-computable mean m_h = E[sech^2(c+P)] (Gauss-
# Hermite over P~N(0,||W_col||^2)); then the correction splits into
#   ehat @ (dW u)   u = m*v   -- linear in the streamed fp8 operand:
#                                folded into the top-8 |v| weight columns
#                                (delta x_h* = alpha/(v m) * ehat@(dW u)),
#                                zero device cost
#   de @ (W u_b)    de = enc - ehat, per-batch m_b = E[sech^2(c_b+P)]
#                             -- host GEMV -> [B,S] map, added to the
#                                scores row by one DVE [1,512] add/block
# Sim-validated: 1.78e-2 (plain vmix0) -> 8.93e-3, vs vmix1's 1.27e-2.
CORR = True
CORR_FOLD_K = 8
# PAIR2: group-major pair body (see _emit_body_vmix_pair2) for kf=0
PAIR2 = True
# FD1024: single FD=1024 matmuls writing 2 PSUM banks (halves instruction
# count).  REJECTED: walrus ISA check (Matmult I-97) enforces the one-bank
# output limit; ACT *reads* across 2 banks are fine (pair2 relies on them)
FD1024 = False
# WSHARE: consecutive matmuls that share one weight tile, probing whether
# the toolchain skips redundant LDWEIGHTS (DR LDW is ~256cyc and appears
# not to overlap the stream).  0 = off; 2 = the two 512-col halves of a
# pair run back-to-back per (g,k2); 4 = two PAIRS processed group-locked
# so each weight tile feeds 4 consecutive matmuls
WSHARE = 4
# HOST_SOFTMAX: device returns corrected raw scores; kernel() applies the
# softmax on host (131K elems, ~0.00005% of the FLOPs -- same precedent as
# the host-side h@W_h+b prep).  Removes all single-lane Exp/rescale ACT+DVE
# ops (~25us engine time) and the exposed end-of-batch tail.
# MEASURED: no win (312.6 vs 308.4us interleaved) -- Exp/rescale were
# already hidden under PE.  Kept off: device softmax, cleaner contract.
HOST_SOFTMAX = False
# PSUM ring split: (scp bufs, epp bufs) -- 2*scp + 2*2*epp must be <= 8.
# Measured interleaved: scp1/epp3 315.6us vs scp2/epp2 332.0us -- the ep
# ring depth (PE runway ahead of ACT) beats de-serializing the scores ring
SC_BUFS = 1
EPP_BUFS = 3
# fp8 operands are pre-scaled into e4m3's normal range (enc x16, W x512;
# raw W_e has std 0.022 and would quantize half its mass subnormally);
# the 1/8192 descale folds into the tanh activation's scale immediate.
E8_SCALE = 16.0
W8_SCALE = 512.0
# v-dot accumulator dtype knob (f16 halves DVE SBUF traffic; error +5e-6)
ACC_F16 = True
# tanh-output dtype (16-bit keeps DVE in its 2x packed mode; f16 beats bf16
# in precision for values in [-1,1])
TH_DT = "f16" 
# v-dot on DVE ("dvesc" mode) keeps v in f32
VW_F32 = True
F8 = mybir.dt.float8e4
KT2 = KT // 2  # 256-deep contraction tiles for DoubleRow

_NC_CACHE = {}


def _mm_dt():
    return BF16 if MM_DTYPE == "bf16" else F32R


def _emit_body(nc, pools, params, batches=None, mode="full"):
    AFT = mybir.ActivationFunctionType
    enc_pool, th_pool, soft_pool, ep_pool, sc_pool = pools
    encT, out, w_sb, v_sb, c_sb, ones_sb, et_shared, wh, lazy_w = params[:9]
    hyb = "hyb" in mode
    if hyb:
        encT8, w8_sb = params[9:]
    batches = list(range(BPC)) if batches is None else batches
    fp8 = MM_DTYPE == "fp8dr"
    th_dt = F32 if "dvesc" in mode else _mm_dt()

    # flat list of (batch, s-block); scores finalization for block i is
    # deferred into block i+1 so PE never waits on ACT/DVE results
    blocks = [(b, isb) for b in batches for isb in range(NSB)]
    soft = {}    # b -> (exp_row, parts)
    deferred = None  # (b, isb, sc_or_acc, ths)

    def finish_block(dfr):
        b, isb, acc, ths = dfr
        exp_row, parts = soft[b]
        if "dvesc" in mode:
            sc = sc_pool.tile([1, SB], F32, tag="sc")
            nc.tensor.matmul(sc, ones_sb, acc, start=True, stop=True)
        else:
            sc = sc_pool.tile([1, SB], F32, tag="sc")
            for hd in range(HT):
                nc.tensor.matmul(sc, v_sb[:, hd:hd + 1], ths[hd],
                                 start=(hd == 0), stop=(hd == HT - 1))
        nc.scalar.activation(
            exp_row[:, isb * SB:(isb + 1) * SB], sc, AFT.Exp,
            accum_out=parts[:, isb:isb + 1])
        if isb == NSB - 1:
            # batch done: softmax normalization + output
            tot = soft_pool.tile([1, 1], F32, tag="tot")
            nc.vector.tensor_reduce(tot, parts, axis=mybir.AxisListType.X,
                                    op=mybir.AluOpType.add)
            rinv = soft_pool.tile([1, 1], F32, tag="rinv")
            nc.vector.reciprocal(rinv, tot)
            # scale on ScalarE (1.2 GHz vs DVE 0.96 single-lane), in two
            # chunks so the first chunk's output DMA overlaps the second
            # chunk's scaling -- trims the exposed final-batch tail
            half = S // 2
            for c2 in range(2):
                oc = soft_pool.tile([1, half], F32, tag="oc", bufs=4,
                                    name=f"oc_{b}_{c2}")
                nc.scalar.activation(oc, exp_row[:, c2 * half:(c2 + 1) * half],
                                     AFT.Copy, scale=rinv)
                nc.sync.dma_start(out=out[b:b + 1, c2 * half:(c2 + 1) * half],
                                  in_=oc)
            del soft[b]

    for b, isb in blocks:
        if b not in soft:
            soft[b] = (soft_pool.tile([1, S], F32, tag="exp_row",
                                      name=f"exp_row_{b}"),
                       soft_pool.tile([1, NSB], F32, tag="parts",
                                      name=f"parts_{b}"))
        if "compute" in mode:
            et = et_shared
        elif hyb:
            et8 = enc_pool.tile([128, 2, SB], F8, tag="et8")
            nc.sync.dma_start(
                out=et8, in_=encT8[b, :, :, isb * SB:(isb + 1) * SB])
            et = []
            for k in range(KT - 2):
                t = enc_pool.tile([128, SB], F32R, tag="et")
                nc.sync.dma_start(
                    out=t,
                    in_=encT[b, k * 128:(k + 1) * 128, isb * SB:(isb + 1) * SB])
                et.append(t)
        elif fp8:
            et = []
            for k2 in range(KT2):
                t = enc_pool.tile([128, 2, SB], F8, tag="et")
                nc.sync.dma_start(
                    out=t, in_=encT[b, k2, :, :, isb * SB:(isb + 1) * SB])
                et.append(t)
        elif "wet" in mode:
            # wide et: one [128, 2*SB] tile per k covers two s-blocks --
            # halves DMA count and first-use sem waits
            first = (b, isb) == blocks[0]
            if isb % 2 == 0:
                etw = []
                for k in range(KT):
                    t = enc_pool.tile([128, 2 * SB], _mm_dt(), tag="etw",
                                      bufs=12, name=f"etw{k}")
                    nc.sync.dma_start(
                        out=t,
                        in_=encT[b, k * 128:(k + 1) * 128,
                                 isb * SB:(isb + 2) * SB])
                    etw.append(t)
                    if lazy_w and first:
                        ck = HT * 128
                        nc.sync.dma_start(out=w_sb[:, k * ck:(k + 1) * ck],
                                          in_=wh[:, k * ck:(k + 1) * ck])
                _emit_body.etw = etw
            off = (isb % 2) * SB
            et = [t[:, off:off + SB] for t in _emit_body.etw]
        else:
            first = (b, isb) == blocks[0]
            et = []
            for k in range(KT):
                t = enc_pool.tile([128, SB], _mm_dt(), tag="et")
                nc.sync.dma_start(
                    out=t,
                    in_=encT[b, k * 128:(k + 1) * 128, isb * SB:(isb + 1) * SB])
                et.append(t)
                if lazy_w and first:
                    ck = HT * 128
                    nc.sync.dma_start(out=w_sb[:, k * ck:(k + 1) * ck],
                                      in_=wh[:, k * ck:(k + 1) * ck])

        acc = None
        ths = []
        if "g1" in mode:
            first_blk = (b, isb) == blocks[0]
            last_blk = (b, isb) == blocks[-1]
            ep = ep_pool.tile([128, SB], F32, tag="ep1", bufs=1,
                              name="ep_g1")
            for hd in range(HT):
                for k in range(KT):
                    w_tile = w_sb[:, (k * HT + hd) * 128:(k * HT + hd + 1) * 128]
                    nc.tensor.matmul(
                        ep, w_tile, et[k],
                        start=(first_blk and hd == 0 and k == 0),
                        stop=(last_blk and hd == HT - 1 and k == KT - 1))
            if last_blk:
                probe = soft_pool.tile([128, 1], F32, tag="probe")
                nc.scalar.activation(probe, ep[:, 0:1], AFT.Copy)
            continue
        for hd in range(HT):
            ep = ep_pool.tile([128, SB], F32, tag="ep")
            if fp8:
                for k2 in range(KT2):
                    nc.tensor.matmul(
                        ep, w_sb[:, k2 * HT + hd, :, :], et[k2],
                        start=(k2 == 0), stop=(k2 == KT2 - 1),
                        perf_mode=mybir.MatmulPerfMode.DoubleRow)
            elif hyb:
                nc.tensor.matmul(ep, w8_sb[:, hd, :, :], et8,
                                 start=True, stop=False,
                                 perf_mode=mybir.MatmulPerfMode.DoubleRow)
                for k in range(KT - 2):
                    w_tile = w_sb[:, (k * HT + hd) * 128:(k * HT + hd + 1) * 128]
                    nc.tensor.matmul(ep, w_tile, et[k],
                                     start=False, stop=(k == KT - 3))
            else:
                kr = range(KT // 2) if "k4" in mode else range(KT)
                for k in kr:
                    w_tile = w_sb[:, (k * HT + hd) * 128:(k * HT + hd + 1) * 128]
                    nc.tensor.matmul(ep, w_tile, et[k],
                                     start=(k == 0),
                                     stop=(k == list(kr)[-1]))
            if hd == 1 and deferred is not None:
                # PE work for the previous block's scores goes here, long
                # after its inputs are ready
                finish_block(deferred)
                deferred = None
            if "noact" in mode:
                if hd == HT - 1:
                    probe = soft_pool.tile([128, 1], F32, tag="probe")
                    nc.scalar.activation(probe, ep[:, 0:1], AFT.Copy)
                continue
            th = th_pool.tile([128, SB], th_dt, tag="th")
            nc.scalar.activation(
                th, ep, AFT.Tanh, bias=c_sb[:, b * HT + hd: b * HT + hd + 1])
            ths.append(th)
            if "dvesc" in mode:
                if hd == 0:
                    acc = th_pool.tile([128, SB], F32, tag="acc", bufs=3)
                    nc.vector.tensor_scalar_mul(acc, th, v_sb[:, 0:1])
                else:
                    nc.vector.scalar_tensor_tensor(
                        acc, th, v_sb[:, hd:hd + 1], acc,
                        op0=mybir.AluOpType.mult, op1=mybir.AluOpType.add)
        if "noact" in mode:
            continue
        if "dvesc" in mode:
            acc8 = th_pool.tile([128, SB], F32R, tag="acc8", bufs=3)
            nc.scalar.activation(acc8, acc, AFT.Copy)
            acc = acc8
        deferred = (b, isb, acc if "dvesc" in mode else None, ths)
    if deferred is not None and "noact" not in mode:
        finish_block(deferred)


def _emit_body_vmix(nc, pools, params, batches=None, kf=2, sw=False):
    """v-sorted mixed precision: hd-tiles [0,kf) bf16, [kf,HT) fp8e4-DR."""
    AFT = mybir.ActivationFunctionType
    enc_pool, th_pool, soft_pool, ep_pool, sc_pool = pools
    (encTb, encT8, out, wb_sb, w8_sb, v_sb, c_sb, ones_sb,
     whb, wh8, lazy_w, corr_d) = params
    HF = HT - kf
    batches = list(range(BPC)) if batches is None else batches
    blocks = [(b, isb) for b in batches for isb in range(NSB)]
    soft = {}
    deferred = None  # (b, isb, acc8, corrt)

    def finish_block(dfr):
        b, isb, acc, corrt = dfr
        exp_row, parts = soft[b]
        sc = sc_pool.tile([1, SB], F32, tag="sc")
        nc.tensor.matmul(sc, ones_sb, acc, start=True, stop=True)
        if corrt is not None:
            sc2 = soft_pool.tile([1, SB], F32, tag="sc2", bufs=4)
            nc.vector.tensor_tensor(
                sc2, sc, corrt, op=mybir.AluOpType.add)
            sc = sc2
        nc.scalar.activation(
            exp_row[:, isb * SB:(isb + 1) * SB], sc, AFT.Exp,
            accum_out=parts[:, isb:isb + 1])
        if isb == NSB - 1:
            tot = soft_pool.tile([1, 1], F32, tag="tot")
            nc.vector.tensor_reduce(tot, parts, axis=mybir.AxisListType.X,
                                    op=mybir.AluOpType.add)
            rinv = soft_pool.tile([1, 1], F32, tag="rinv")
            nc.vector.reciprocal(rinv, tot)
            half = S // 2
            for c2 in range(2):
                oc = soft_pool.tile([1, half], F32, tag="oc", bufs=4,
                                    name=f"oc_{b}_{c2}")
                nc.scalar.activation(oc, exp_row[:, c2 * half:(c2 + 1) * half],
                                     AFT.Copy, scale=rinv)
                nc.sync.dma_start(out=out[b:b + 1, c2 * half:(c2 + 1) * half],
                                  in_=oc)
            del soft[b]

    for b, isb in blocks:
        if b not in soft:
            soft[b] = (soft_pool.tile([1, S], F32, tag="exp_row",
                                      name=f"exp_row_{b}"),
                       soft_pool.tile([1, NSB], F32, tag="parts",
                                      name=f"parts_{b}"))
        first = (b, isb) == blocks[0]
        corrt = None
        if corr_d is not None:
            corrt = soft_pool.tile([1, SB], F32, tag="corrt", bufs=4)
            nc.sync.dma_start(
                out=corrt,
                in_=corr_d[:, b * S + isb * SB:b * S + (isb + 1) * SB])
        etb = []
        for k in range(KT):
            if kf == 0:
                break
            t = enc_pool.tile([128, SB], BF16, tag="etb", bufs=18)
            nc.sync.dma_start(
                out=t,
                in_=encTb[b, k * 128:(k + 1) * 128, isb * SB:(isb + 1) * SB])
            etb.append(t)
            if lazy_w and first and k < kf:
                ck = KT * 128
                nc.sync.dma_start(out=wb_sb[:, k * ck:(k + 1) * ck],
                                  in_=whb[:, k * ck:(k + 1) * ck])
        et8 = []
        for k2 in range(KT2):
            if kf == HT:
                break
            t = enc_pool.tile([128, 2, SB], F8, tag="et8", bufs=10)
            nc.sync.dma_start(
                out=t, in_=encT8[b, k2, :, :, isb * SB:(isb + 1) * SB])
            et8.append(t)
            if lazy_w and first:
                nc.sync.dma_start(out=w8_sb[:, k2 * HF:(k2 + 1) * HF, :, :],
                                  in_=wh8[:, k2 * HF:(k2 + 1) * HF, :, :])

        acc = None
        for g in range(HT):
            ep = ep_pool.tile([128, SB], F32, tag="ep")
            if g < kf:
                for k in range(KT):
                    w_tile = wb_sb[:, (g * KT + k) * 128:(g * KT + k + 1) * 128]
                    nc.tensor.matmul(ep, w_tile, etb[k],
                                     start=(k == 0), stop=(k == KT - 1))
                th_scale = 1.0
            else:
                hf = g - kf
                pm = (mybir.MatmulPerfMode.DoubleRowSwInterleave if sw
                      else mybir.MatmulPerfMode.DoubleRow)
                for k2 in range(KT2):
                    nc.tensor.matmul(
                        ep, w8_sb[:, k2 * HF + hf, :, :], et8[k2],
                        start=(k2 == 0), stop=(k2 == KT2 - 1),
                        perf_mode=pm)
                th_scale = 1.0 / (E8_SCALE * W8_SCALE)
            if g == 4 and deferred is not None:
                # previous block's scores matmul goes here, long after its
                # inputs are ready, so PE never stalls on ACT/DVE
                finish_block(deferred)
                deferred = None
            th = th_pool.tile([128, SB],
                              F16 if TH_DT == "f16" else BF16, tag="th")
            nc.scalar.activation(
                th, ep, AFT.Tanh, scale=th_scale,
                bias=c_sb[:, b * HT + g: b * HT + g + 1])
            acc_dt = F16 if ACC_F16 else F32
            lst_dt = F16 if ACC_F16 else F32R
            if g == 0:
                acc = th_pool.tile([128, SB], acc_dt, tag="acc", bufs=3)
                nc.vector.tensor_scalar_mul(acc, th, v_sb[:, 0:1])
            else:
                # f16 acc halves DVE SBUF traffic; the rounding noise is
                # ~3e-4 of the score scale, invisible next to the fp8 noise.
                # The last tile's store doubles as the scores matmul input.
                nxt = acc if g < HT - 1 else th_pool.tile(
                    [128, SB], lst_dt, tag="accr", bufs=3)
                nc.vector.scalar_tensor_tensor(
                    nxt, th, v_sb[:, g:g + 1], acc,
                    op0=mybir.AluOpType.mult, op1=mybir.AluOpType.add)
                acc = nxt
        deferred = (b, isb, acc, corrt)
    if deferred is not None:
        finish_block(deferred)


def _emit_body_vmix_pair(nc, pools, params, batches=None, kf=1, sw=False):
    """vmix over s-block PAIRS: th/acc span 2 blocks (1024 cols) so each
    DVE v-dot op amortizes its ~151-cycle overhead over 2x the elements,
    and et DMAs go 2 blocks wide (half the DMA count / matmul sem waits)."""
    AFT = mybir.ActivationFunctionType
    enc_pool, th_pool, soft_pool, ep_pool, sc_pool = pools
    (encTb, encT8, out, wb_sb, w8_sb, v_sb, c_sb, ones_sb,
     whb, wh8, lazy_w, corr_d) = params
    HF = HT - kf
    SB2 = 2 * SB
    batches = list(range(BPC)) if batches is None else batches
    pairs = [(b, ip) for b in batches for ip in range(NSB // 2)]
    soft = {}
    deferred = None  # (b, ip, accr_pair)

    def finish_pair(dfr):
        b, ip, accr_halves, corrt = dfr
        exp_row, parts = soft[b]
        for half in range(2):
            isb = 2 * ip + half
            sc = sc_pool.tile([1, SB], F32, tag="sc")
            nc.tensor.matmul(sc, ones_sb, accr_halves[half],
                             start=True, stop=True)
            if corrt is not None:
                sc2 = soft_pool.tile([1, SB], F32, tag="sc2", bufs=4)
                nc.vector.tensor_tensor(
                    sc2, sc, corrt[:, half * SB:(half + 1) * SB],
                    op=mybir.AluOpType.add)
                sc = sc2
            nc.scalar.activation(
                exp_row[:, isb * SB:(isb + 1) * SB], sc, AFT.Exp,
                accum_out=parts[:, isb:isb + 1])
        if 2 * ip + 1 == NSB - 1:
            tot = soft_pool.tile([1, 1], F32, tag="tot")
            nc.vector.tensor_reduce(tot, parts, axis=mybir.AxisListType.X,
                                    op=mybir.AluOpType.add)
            rinv = soft_pool.tile([1, 1], F32, tag="rinv")
            nc.vector.reciprocal(rinv, tot)
            # 4 chunks: each [1,1024] ACT op is ~0.85us, so the rescale
            # never head-of-line-blocks the tanh stream for long, and each
            # chunk's output DMA overlaps the next chunk's scaling
            qt = S // 4
            for c4 in range(4):
                oc = soft_pool.tile([1, qt], F32, tag="oc", bufs=8,
                                    name=f"oc_{b}_{c4}")
                nc.scalar.activation(oc, exp_row[:, c4 * qt:(c4 + 1) * qt],
                                     AFT.Copy, scale=rinv)
                nc.sync.dma_start(out=out[b:b + 1, c4 * qt:(c4 + 1) * qt],
                                  in_=oc)
            del soft[b]

    for b, ip in pairs:
        if b not in soft:
            soft[b] = (soft_pool.tile([1, S], F32, tag="exp_row",
                                      name=f"exp_row_{b}"),
                       soft_pool.tile([1, NSB], F32, tag="parts",
                                      name=f"parts_{b}"))
        first = (b, ip) == pairs[0]
        last = (b, ip) == pairs[-1]
        accr_h = []
        s0 = 2 * ip * SB
        corrt = None
        if corr_d is not None:
            corrt = soft_pool.tile([1, SB2], F32, tag="corrt", bufs=4)
            nc.sync.dma_start(out=corrt,
                              in_=corr_d[:, b * S + s0:b * S + s0 + SB2])
        etb, et8 = [], []
        for k in range(KT):
            if kf == 0:
                break
            t = enc_pool.tile([128, SB2], BF16, tag="etb", bufs=24)
            nc.sync.dma_start(
                out=t, in_=encTb[b, k * 128:(k + 1) * 128, s0:s0 + SB2])
            etb.append(t)
            if lazy_w and first and k < kf:
                ck = KT * 128
                nc.sync.dma_start(out=wb_sb[:, k * ck:(k + 1) * ck],
                                  in_=whb[:, k * ck:(k + 1) * ck])
        for k2 in range(KT2):
            if kf == HT:
                break
            t = enc_pool.tile([128, 2, SB2], F8, tag="et8", bufs=12)
            nc.sync.dma_start(out=t, in_=encT8[b, k2, :, :, s0:s0 + SB2])
            et8.append(t)
            if lazy_w and first:
                nc.sync.dma_start(out=w8_sb[:, k2 * HF:(k2 + 1) * HF, :, :],
                                  in_=wh8[:, k2 * HF:(k2 + 1) * HF, :, :])

        ths = {}
        acc = None
        for blk in range(2):
            for g in range(HT):
                ep = ep_pool.tile([128, SB], F32, tag="ep")
                if g < kf:
                    for k in range(KT):
                        w_tile = wb_sb[:, (g * KT + k) * 128:
                                       (g * KT + k + 1) * 128]
                        nc.tensor.matmul(
                            ep, w_tile, etb[k][:, blk * SB:(blk + 1) * SB],
                            start=(k == 0), stop=(k == KT - 1))
                    th_scale = 1.0
                else:
                    hf = g - kf
                    pm = (mybir.MatmulPerfMode.DoubleRowSwInterleave if sw
                          else mybir.MatmulPerfMode.DoubleRow)
                    for k2 in range(KT2):
                        nc.tensor.matmul(
                            ep, w8_sb[:, k2 * HF + hf, :, :],
                            et8[k2][:, :, blk * SB:(blk + 1) * SB],
                            start=(k2 == 0), stop=(k2 == KT2 - 1),
                            perf_mode=pm)
                    th_scale = 1.0 / (E8_SCALE * W8_SCALE)
                if blk == 1 and g == 0 and deferred is not None:
                    # consume the previous pair's accr as late as possible:
                    # its DVE chain is ~10us of engine-serial work that only
                    # starts in blk1, so at blk0-g4 the scores matmul could
                    # still be waiting on it under load
                    finish_pair(deferred)
                    deferred = None
                if blk == 0:
                    ths[g] = th_pool.tile([128, SB2], F16, tag=f"th{g}",
                                          bufs=2, name=f"th{g}")
                th = ths[g]
                nc.scalar.activation(
                    th[:, blk * SB:(blk + 1) * SB], ep, AFT.Tanh,
                    scale=th_scale,
                    bias=c_sb[:, b * HT + g: b * HT + g + 1])
                if last:
                    # last pair: per-half v-dot so blk0's chain hides under
                    # blk1's matmuls -- the pair-wide chain would otherwise
                    # run ~10us serially exposed after the final matmul
                    ths_b = th[:, blk * SB:(blk + 1) * SB]
                    if g == 0:
                        acc = th_pool.tile([128, SB], F16, tag=f"acch{blk}",
                                           bufs=1, name=f"acch{blk}")
                        nc.vector.tensor_scalar_mul(acc, ths_b, v_sb[:, 0:1])
                    else:
                        nxt = acc if g < HT - 1 else th_pool.tile(
                            [128, SB], F16, tag=f"accrh{blk}", bufs=1,
                            name=f"accrh{blk}")
                        nc.vector.scalar_tensor_tensor(
                            nxt, ths_b, v_sb[:, g:g + 1], acc,
                            op0=mybir.AluOpType.mult,
                            op1=mybir.AluOpType.add)
                        acc = nxt
                        if g == HT - 1:
                            accr_h.append(acc)
                elif blk == 1:
                    # v-dot over the full pair right after its second half
                    if g == 0:
                        acc = th_pool.tile([128, SB2], F16, tag="accp",
                                           bufs=2)
                        nc.vector.tensor_scalar_mul(acc, th, v_sb[:, 0:1])
                    else:
                        nxt = acc if g < HT - 1 else th_pool.tile(
                            [128, SB2], F16, tag="accrp", bufs=3)
                        nc.vector.scalar_tensor_tensor(
                            nxt, th, v_sb[:, g:g + 1], acc,
                            op0=mybir.AluOpType.mult,
                            op1=mybir.AluOpType.add)
                        acc = nxt
        if last:
            deferred = (b, ip, accr_h, corrt)
        else:
            deferred = (b, ip, [acc[:, 0:SB], acc[:, SB:2 * SB]], corrt)
    if deferred is not None:
        finish_pair(deferred)


def _emit_body_vmix_pair2(nc, pools, params, batches=None, kf=0, sw=False,
                          probe=()):
    """all-fp8 pair path, group-major: per group g one [128, 2*SB] 2-bank
    PSUM tile is filled by two 4-matmul DR chains (one per 512-col half),
    then ONE pair-wide tanh + ONE pair-wide DVE v-dot step.  Halves the
    ACT op count vs block-major (each ACTIVATE pays a 352-cycle fixed
    overhead: 16x720ns -> 8x1.15us) -- ACT was co-critical with PE.
    Group-major also lets the DVE acc chain trail each group's tanh, so
    the end-of-stream drain is one stt link (~2us), not the whole chain.
    Scores finalization: both halves' ones-matmuls write one [1, 2*SB]
    2-bank PSUM tile; corr-add is one DVE [1,1024] op; one pair-wide Exp
    with per-pair accum_out.  Softmax rescale alternates ACT/DVE chunks
    so neither engine eats the whole tail."""
    AFT = mybir.ActivationFunctionType
    enc_pool, th_pool, soft_pool, ep_pool, sc_pool = pools
    (encTb, encT8, out, wb_sb, w8_sb, v_sb, c_sb, ones_sb,
     whb, wh8, lazy_w, corr_d) = params
    assert kf == 0
    HF = HT
    SB2 = 2 * SB
    th_scale = 1.0 / (E8_SCALE * W8_SCALE)
    pm = (mybir.MatmulPerfMode.DoubleRowSwInterleave if sw
          else mybir.MatmulPerfMode.DoubleRow)
    batches = list(range(BPC)) if batches is None else batches
    pairs = [(b, ip) for b in batches for ip in range(NSB // 2)]
    soft = {}
    deferred = None  # (b, ip, accr, corrt)

    def finish_pair(dfr):
        b, ip, accr, corrt = dfr
        exp_row, parts = soft[b]
        scp = sc_pool.tile([1, SB2], F32, tag="scp", bufs=1)
        if FD1024:
            nc.tensor.matmul(scp, ones_sb, accr, start=True, stop=True)
        else:
            for half in range(2):
                nc.tensor.matmul(scp[:, half * SB:(half + 1) * SB], ones_sb,
                                 accr[:, half * SB:(half + 1) * SB],
                                 start=True, stop=True)
        sc2 = soft_pool.tile([1, SB2], F32, tag="sc2", bufs=3)
        nc.vector.tensor_tensor(sc2, scp, corrt, op=mybir.AluOpType.add)
        nc.scalar.activation(
            exp_row[:, 2 * ip * SB:2 * (ip + 1) * SB], sc2, AFT.Exp,
            accum_out=parts[:, ip:ip + 1])
        if 2 * ip + 1 == NSB - 1:
            tot = soft_pool.tile([1, 1], F32, tag="tot")
            nc.vector.tensor_reduce(tot, parts, axis=mybir.AxisListType.X,
                                    op=mybir.AluOpType.add)
            rinv = soft_pool.tile([1, 1], F32, tag="rinv")
            nc.vector.reciprocal(rinv, tot)
            qt = S // 4
            for c4 in range(4):
                oc = soft_pool.tile([1, qt], F32, tag="oc", bufs=8,
                                    name=f"oc_{b}_{c4}")
                if c4 % 2 == 0:
                    nc.scalar.activation(oc, exp_row[:, c4 * qt:(c4 + 1) * qt],
                                         AFT.Copy, scale=rinv)
                else:
                    nc.vector.tensor_scalar_mul(
                        oc, exp_row[:, c4 * qt:(c4 + 1) * qt], rinv)
                nc.sync.dma_start(out=out[b:b + 1, c4 * qt:(c4 + 1) * qt],
                                  in_=oc)
            del soft[b]

    for b, ip in pairs:
        if b not in soft:
            soft[b] = (soft_pool.tile([1, S], F32, tag="exp_row",
                                      name=f"exp_row_{b}"),
                       soft_pool.tile([1, NSB // 2], F32, tag="parts",
                                      name=f"parts_{b}"))
        first = (b, ip) == pairs[0]
        s0 = 2 * ip * SB
        corrt = None
        if corr_d is not None:
            corrt = soft_pool.tile([1, SB2], F32, tag="corrt", bufs=4)
            nc.sync.dma_start(out=corrt,
                              in_=corr_d[:, b * S + s0:b * S + s0 + SB2])
        et8 = []
        for k2 in range(KT2):
            t = enc_pool.tile([128, 2, SB2], F8, tag="et8", bufs=12)
            nc.sync.dma_start(out=t, in_=encT8[b, k2, :, :, s0:s0 + SB2])
            et8.append(t)
            if lazy_w and first:
                nc.sync.dma_start(out=w8_sb[:, k2 * HF:(k2 + 1) * HF, :, :],
                                  in_=wh8[:, k2 * HF:(k2 + 1) * HF, :, :])

        acc = None
        for g in range(HT):
            ep = ep_pool.tile([128, SB2], F32, tag="epp", bufs=3)
            if WSHARE == 2:
                for k2 in range(KT2):
                    for blk in range(2):
                        nc.tensor.matmul(
                            ep[:, blk * SB:(blk + 1) * SB],
                            w8_sb[:, k2 * HF + g, :, :],
                            et8[k2][:, :, blk * SB:(blk + 1) * SB],
                            start=(k2 == 0), stop=(k2 == KT2 - 1),
                            perf_mode=pm)
            else:
                for blk in range(2):
                    for k2 in range(KT2):
                        nc.tensor.matmul(
                            ep[:, blk * SB:(blk + 1) * SB],
                            w8_sb[:, k2 * HF + g, :, :],
                            et8[k2][:, :, blk * SB:(blk + 1) * SB],
                            start=(k2 == 0), stop=(k2 == KT2 - 1),
                            perf_mode=pm)
            if g == 4 and deferred is not None:
                finish_pair(deferred)
                deferred = None
            if "noact" in probe:
                continue
            th = th_pool.tile([128, SB2], F16, tag=f"th{g}", bufs=2,
                              name=f"th{g}")
            nc.scalar.activation(
                th, ep, AFT.Tanh, scale=th_scale,
                bias=c_sb[:, b * HT + g: b * HT + g + 1])
            if "nodve" in probe:
                continue
            if g == 0:
                acc = th_pool.tile([128, SB2], F16, tag="accp", bufs=2)
                nc.vector.tensor_scalar_mul(acc, th, v_sb[:, 0:1])
            else:
                nxt = acc if g < HT - 1 else th_pool.tile(
                    [128, SB2], F16, tag="accrp", bufs=3)
                nc.vector.scalar_tensor_tensor(
                    nxt, th, v_sb[:, g:g + 1], acc,
                    op0=mybir.AluOpType.mult, op1=mybir.AluOpType.add)
                acc = nxt
        if not ("noact" in probe or "nodve" in probe or "nofin" in probe):
            deferred = (b, ip, acc, corrt)
    if deferred is not None:
        finish_pair(deferred)


def _emit_body_vmix_sp4(nc, pools, params, batches=None, kf=0, sw=False,
                        probe=()):
    """WSHARE=4: two pairs processed group-locked so each (g,k2) weight
    tile feeds 4 consecutive matmuls (pairA-h0, pairA-h1, pairB-h0,
    pairB-h1) -- LDWEIGHTS elision probe.  Otherwise pair2 semantics."""
    AFT = mybir.ActivationFunctionType
    enc_pool, th_pool, soft_pool, ep_pool, sc_pool = pools
    (encTb, encT8, out, wb_sb, w8_sb, v_sb, c_sb, ones_sb,
     whb, wh8, lazy_w, corr_d) = params
    assert kf == 0
    HF = HT
    SB2 = 2 * SB
    th_scale = 1.0 / (E8_SCALE * W8_SCALE)
    pm = (mybir.MatmulPerfMode.DoubleRowSwInterleave if sw
          else mybir.MatmulPerfMode.DoubleRow)
    batches = list(range(BPC)) if batches is None else batches
    sps = [(b, sp) for b in batches for sp in range(NSB // 4)]
    soft = {}
    deferred = []  # [(b, ip, accr, corrt), ...]

    def finish_pair(dfr):
        b, ip, accr, corrt = dfr
        # scp ring of 2: pair i+1's scores matmul (PE) must not wait for
        # pair i's corr-add (DVE, queued behind stts) to release the banks
        scp = sc_pool.tile([1, SB2], F32, tag="scp", bufs=SC_BUFS)
        for half in range(2):
            nc.tensor.matmul(scp[:, half * SB:(half + 1) * SB], ones_sb,
                             accr[:, half * SB:(half + 1) * SB],
                             start=True, stop=True)
        sc2 = soft_pool.tile([1, SB2], F32, tag="sc2", bufs=3)
        nc.vector.tensor_tensor(sc2, scp, corrt, op=mybir.AluOpType.add)
        if HOST_SOFTMAX:
            nc.sync.dma_start(
                out=out[b:b + 1, 2 * ip * SB:2 * (ip + 1) * SB], in_=sc2)
            return
        exp_row, parts = soft[b]
        nc.scalar.activation(
            exp_row[:, 2 * ip * SB:2 * (ip + 1) * SB], sc2, AFT.Exp,
            accum_out=parts[:, ip:ip + 1])
        if 2 * ip + 1 == NSB - 1:
            tot = soft_pool.tile([1, 1], F32, tag="tot")
            nc.vector.tensor_reduce(tot, parts, axis=mybir.AxisListType.X,
                                    op=mybir.AluOpType.add)
            rinv = soft_pool.tile([1, 1], F32, tag="rinv")
            nc.vector.reciprocal(rinv, tot)
            qt = S // 4
            for c4 in range(4):
                oc = soft_pool.tile([1, qt], F32, tag="oc", bufs=8,
                                    name=f"oc_{b}_{c4}")
                # DVE-only: an ACT rescale burst here would delay tanhs
                # and stall PE on the (now 2-deep) ep ring
                nc.vector.tensor_scalar_mul(
                    oc, exp_row[:, c4 * qt:(c4 + 1) * qt], rinv)
                nc.sync.dma_start(out=out[b:b + 1, c4 * qt:(c4 + 1) * qt],
                                  in_=oc)
            del soft[b]

    static = {}
    for b, sp in sps:
        if b not in soft and not HOST_SOFTMAX:
            soft[b] = (soft_pool.tile([1, S], F32, tag="exp_row",
                                      name=f"exp_row_{b}"),
                       soft_pool.tile([1, NSB // 2], F32, tag="parts",
                                      name=f"parts_{b}"))
        first = (b, sp) == sps[0]
        if "nodma" in probe and not first:
            corrts, et8s = static["c"], static["e"]
            ips = (2 * sp, 2 * sp + 1)
        else:
            ips = (2 * sp, 2 * sp + 1)
            corrts, et8s = [], []
            _dma_ips = ips
        for ip in (() if ("nodma" in probe and not first) else ips):
            s0 = 2 * ip * SB
            corrt = None
            if corr_d is not None:
                corrt = soft_pool.tile([1, SB2], F32, tag="corrt", bufs=6)
                nc.sync.dma_start(out=corrt,
                                  in_=corr_d[:, b * S + s0:b * S + s0 + SB2])
            corrts.append(corrt)
            et8 = []
            for k2 in range(KT2):
                t = enc_pool.tile([128, 2, SB2], F8, tag="et8", bufs=16)
                nc.sync.dma_start(out=t, in_=encT8[b, k2, :, :, s0:s0 + SB2])
                et8.append(t)
                if lazy_w and first and ip == ips[0]:
                    nc.sync.dma_start(
                        out=w8_sb[:, k2 * HF:(k2 + 1) * HF, :, :],
                        in_=wh8[:, k2 * HF:(k2 + 1) * HF, :, :])
            et8s.append(et8)
        if "nodma" in probe and first:
            static["c"], static["e"] = corrts, et8s

        accs = [None, None]
        for g in range(HT):
            eps = [ep_pool.tile([128, SB2], F32, tag="epp", bufs=EPP_BUFS,
                                name=f"epp{pi}")
                   for pi in range(2)]
            for k2 in range(KT2):
                for pi in range(2):
                    for blk in range(2):
                        nc.tensor.matmul(
                            eps[pi][:, blk * SB:(blk + 1) * SB],
                            w8_sb[:, k2 * HF + g, :, :],
                            et8s[pi][k2][:, :, blk * SB:(blk + 1) * SB],
                            start=(k2 == 0), stop=(k2 == KT2 - 1),
                            perf_mode=pm)
            if g in (2, 5) and deferred:
                finish_pair(deferred.pop(0))
            if "noact" in probe:
                continue
            for pi in range(2):
                th = th_pool.tile([128, SB2], F16, tag=f"th{g}", bufs=3,
                                  name=f"th{g}")
                nc.scalar.activation(
                    th, eps[pi], AFT.Tanh, scale=th_scale,
                    bias=c_sb[:, b * HT + g: b * HT + g + 1])
                if "nodve" in probe:
                    continue
                if g == 0:
                    acc0 = th_pool.tile([128, SB2], F16, tag="accp",
                                        bufs=3, name=f"accp{pi}")
                    accs[pi] = acc0
                    nc.vector.tensor_scalar_mul(accs[pi], th, v_sb[:, 0:1])
                else:
                    nxt = accs[pi] if g < HT - 1 else th_pool.tile(
                        [128, SB2], F16, tag="accrp", bufs=4,
                        name=f"accrp{pi}")
                    nc.vector.scalar_tensor_tensor(
                        nxt, th, v_sb[:, g:g + 1], accs[pi],
                        op0=mybir.AluOpType.mult, op1=mybir.AluOpType.add)
                    accs[pi] = nxt
        if not ("noact" in probe or "nodve" in probe or "nofin" in probe):
            for pi in range(2):
                deferred.append((b, ips[pi], accs[pi], corrts[pi]))
    while deferred:
        finish_pair(deferred.pop(0))


def _parse_vmix(mode):
    s = mode[4:]
    pair = s.endswith("p")
    if pair:
        s = s[:-1]
    sw = s.endswith("sw")
    return int(s[:-2] if sw else s), sw, pair


def build_nc(n_loop=1, batches=None, mode=None, ep_bufs=None):
    if mode is None:
        mode = DEFAULT_MODE
    if mode.startswith("vmix"):
        kf, sw, pair = _parse_vmix(mode)
        if ep_bufs is None:
            return build_nc_vmix(n_loop, batches, kf, sw=sw, pair=pair)
        return build_nc_vmix(n_loop, batches, kf, ep_bufs, sw, pair)
    if ep_bufs is None:
        ep_bufs = 4
    key = (MM_DTYPE, n_loop, tuple(batches) if batches else None, mode, ep_bufs)
    if key in _NC_CACHE:
        return _NC_CACHE[key]
    nc = bacc.Bacc(trn_type="TRN2", target_bir_lowering=False, debug=False,
                   num_devices=N_CORES)
    if "hyb" in mode:
        encT = nc.declare_dram_parameter("encT", [BPC, He - 256, S], F32R,
                                         isOutput=False)
        wh = nc.declare_dram_parameter("wh", [128, (KT - 2) * HT * 128], F32R,
                                       isOutput=False)
        encT8 = nc.declare_dram_parameter("encT8", [BPC, 128, 2, S], F8,
                                          isOutput=False)
        wh8 = nc.declare_dram_parameter("wh8", [128, HT, 2, 128], F8,
                                        isOutput=False)
    elif MM_DTYPE == "fp8dr":
        encT = nc.declare_dram_parameter("encT", [BPC, KT2, 128, 2, S], F8,
                                         isOutput=False)
        wh = nc.declare_dram_parameter("wh", [128, KT2 * HT, 2, 128], F8,
                                       isOutput=False)
        encT8 = wh8 = None
    else:
        encT = nc.declare_dram_parameter("encT", [BPC, He, S], _mm_dt(),
                                         isOutput=False)
        wh = nc.declare_dram_parameter("wh", [128, KT * HT * 128], _mm_dt(),
                                       isOutput=False)
    cb = nc.declare_dram_parameter("cb", [128, BPC * HT], F32, isOutput=False)
    vdt = F32 if "dvesc" in mode else _mm_dt()
    vw = nc.declare_dram_parameter("vw", [128, HT], vdt, isOutput=False)
    onesp = nc.declare_dram_parameter("ones", [128, 1], F32R, isOutput=False)
    out = nc.declare_dram_parameter("out", [BPC, S], F32, isOutput=True)

    with tile.TileContext(nc) as tc:
        with (
            tc.tile_pool(name="consts", bufs=1) as consts,
            tc.tile_pool(name="enc", bufs=24) as enc_pool,
            tc.tile_pool(name="th", bufs=10) as th_pool,
            tc.tile_pool(name="soft", bufs=2) as soft_pool,
            tc.tile_pool(name="ep", bufs=ep_bufs, space="PSUM") as ep_pool,
            tc.tile_pool(name="sc", bufs=2, space="PSUM") as sc_pool,
        ):
            lazy_w = n_loop == 1 and MM_DTYPE != "fp8dr" and "hyb" not in mode
            w8_sb = None
            if "hyb" in mode:
                w8_sb = consts.tile([128, HT, 2, 128], F8)
                nc.sync.dma_start(out=w8_sb, in_=wh8[:])
            if "hyb" in mode:
                w_sb = consts.tile([128, (KT - 2) * HT * 128], F32R)
                nc.sync.dma_start(out=w_sb, in_=wh[:])
            elif MM_DTYPE == "fp8dr":
                w_sb = consts.tile([128, KT2 * HT, 2, 128], F8)
                for k2 in range(KT2):
                    nc.sync.dma_start(out=w_sb[:, k2 * HT:(k2 + 1) * HT, :, :],
                                      in_=wh[:, k2 * HT:(k2 + 1) * HT, :, :])
            else:
                w_sb = consts.tile([128, KT * HT * 128], _mm_dt())
                if not lazy_w:
                    ck = HT * 128
                    for k in range(KT):
                        nc.sync.dma_start(out=w_sb[:, k * ck:(k + 1) * ck],
                                          in_=wh[:, k * ck:(k + 1) * ck])
            v_sb = consts.tile([128, HT], vdt)
            nc.sync.dma_start(out=v_sb, in_=vw[:])
            ones_sb = consts.tile([128, 1], F32R)
            nc.sync.dma_start(out=ones_sb, in_=onesp[:])
            c_sb = consts.tile([128, BPC * HT], F32)
            nc.sync.dma_start(out=c_sb, in_=cb[:])

            pools = (enc_pool, th_pool, soft_pool, ep_pool, sc_pool)
            et_shared = None
            if "compute" in mode:
                et_shared = []
                for k in range(KT):
                    t = consts.tile([128, SB], _mm_dt(), tag=f"etc{k}")
                    nc.sync.dma_start(out=t, in_=encT[0, k * 128:(k + 1) * 128, 0:SB])
                    et_shared.append(t)
            params = (encT, out, w_sb, v_sb, c_sb, ones_sb, et_shared,
                      wh, lazy_w)
            if "hyb" in mode:
                params = params + (encT8, w8_sb)
            if n_loop == 1:
                _emit_body(nc, pools, params, batches, mode)
            else:
                with tc.For_i(0, n_loop, 1):
                    _emit_body(nc, pools, params, batches, mode)
    nc.compile()
    _NC_CACHE[key] = nc
    return nc


def build_nc_vmix(n_loop=1, batches=None, kf=2, ep_bufs=6, sw=False,
                  pair=False, probe=()):
    use_corr = CORR and kf == 0
    key = ("vmix", kf, sw, pair, ACC_F16, TH_DT, use_corr, PAIR2, WSHARE, HOST_SOFTMAX, SC_BUFS, EPP_BUFS,
           tuple(probe), n_loop,
           tuple(batches) if batches else None, ep_bufs)
    if key in _NC_CACHE:
        return _NC_CACHE[key]
    HF = HT - kf
    nc = bacc.Bacc(trn_type="TRN2", target_bir_lowering=False, debug=False,
                   num_devices=N_CORES)
    encTb = encT8 = whb = wh8 = corr_d = None
    if kf > 0:
        encTb = nc.declare_dram_parameter("encTb", [BPC, He, S], BF16,
                                          isOutput=False)
        whb = nc.declare_dram_parameter("whb", [128, KT * kf * 128], BF16,
                                        isOutput=False)
    if kf < HT:
        encT8 = nc.declare_dram_parameter("encT8", [BPC, KT2, 128, 2, S], F8,
                                          isOutput=False)
        wh8 = nc.declare_dram_parameter("wh8", [128, KT2 * HF, 2, 128], F8,
                                        isOutput=False)
    if use_corr:
        corr_d = nc.declare_dram_parameter("corr", [1, BPC * S], F32,
                                           isOutput=False)
    cb = nc.declare_dram_parameter("cb", [128, BPC * HT], F32, isOutput=False)
    vw = nc.declare_dram_parameter("vw", [128, HT], F32, isOutput=False)
    onesp = nc.declare_dram_parameter("ones", [128, 1],
                                      F16 if ACC_F16 else F32R,
                                      isOutput=False)
    out = nc.declare_dram_parameter("out", [BPC, S], F32, isOutput=True)

    with tile.TileContext(nc) as tc:
        with (
            tc.tile_pool(name="consts", bufs=1) as consts,
            tc.tile_pool(name="enc", bufs=18) as enc_pool,
            tc.tile_pool(name="th", bufs=10) as th_pool,
            tc.tile_pool(name="soft", bufs=2) as soft_pool,
            tc.tile_pool(name="ep", bufs=ep_bufs, space="PSUM") as ep_pool,
            tc.tile_pool(name="sc", bufs=2, space="PSUM") as sc_pool,
        ):
            lazy_w = n_loop == 1
            wb_sb = w8_sb = None
            if kf > 0:
                wb_sb = consts.tile([128, KT * kf * 128], BF16)
                if not lazy_w:
                    ck = KT * 128
                    for g in range(kf):
                        nc.sync.dma_start(out=wb_sb[:, g * ck:(g + 1) * ck],
                                          in_=whb[:, g * ck:(g + 1) * ck])
            if kf < HT:
                w8_sb = consts.tile([128, KT2 * HF, 2, 128], F8)
                if not lazy_w:
                    for k2 in range(KT2):
                        nc.sync.dma_start(
                            out=w8_sb[:, k2 * HF:(k2 + 1) * HF, :, :],
                            in_=wh8[:, k2 * HF:(k2 + 1) * HF, :, :])
            v_sb = consts.tile([128, HT], F32)
            nc.sync.dma_start(out=v_sb, in_=vw[:])
            ones_sb = consts.tile([128, 1], F16 if ACC_F16 else F32R)
            nc.sync.dma_start(out=ones_sb, in_=onesp[:])
            c_sb = consts.tile([128, BPC * HT], F32)
            nc.sync.dma_start(out=c_sb, in_=cb[:])
            pools = (enc_pool, th_pool, soft_pool, ep_pool, sc_pool)
            params = (encTb, encT8, out, wb_sb, w8_sb, v_sb, c_sb, ones_sb,
                      whb, wh8, lazy_w, corr_d)
            if pair and kf == 0 and PAIR2:
                body = (_emit_body_vmix_sp4 if WSHARE == 4
                        else _emit_body_vmix_pair2)

                def emit(nc, pools, params, batches, kf, sw):
                    body(nc, pools, params, batches, kf, sw, probe)
            else:
                emit = _emit_body_vmix_pair if pair else _emit_body_vmix
            if n_loop == 1:
                emit(nc, pools, params, batches, kf, sw)
            else:
                with tc.For_i(0, n_loop, 1):
                    emit(nc, pools, params, batches, kf, sw)
    nc.compile()
    _NC_CACHE[key] = nc
    return nc


def _gh_mean_tanhp(c, sigma, n=33):
    """m(c,sigma) = E[sech^2(c + sigma*Z)], Z~N(0,1), via Gauss-Hermite."""
    x, w = np.polynomial.hermite_e.hermegauss(n)
    w = (w / w.sum()).astype(np.float64)
    vals = 0.0
    for xi, wi in zip(x, w):
        t = np.tanh(c + sigma * xi)
        vals = vals + wi * (1.0 - t * t)
    return np.asarray(vals, dtype=np.float32)


def prepare_in_maps_vmix(hidden, encoder_outputs, W_attn, b_attn, v_w, kf=2,
                         sw=False):
    import ml_dtypes
    E4 = ml_dtypes.float8_e4m3
    BF = ml_dtypes.bfloat16
    HF = HT - kf
    hidden = np.asarray(hidden, dtype=np.float32)
    enc = np.asarray(encoder_outputs, dtype=np.float32)
    W_attn = np.asarray(W_attn, dtype=np.float32)
    b_attn = np.asarray(b_attn, dtype=np.float32)
    v_w = np.asarray(v_w, dtype=np.float32)

    h = hidden[-1]
    W_h = W_attn[:He]
    W_e = W_attn[He:]
    c = (h @ W_h + b_attn).astype(np.float32)       # [B, Hd]

    # sort h-columns by |v| descending; the reduction over h right after
    # tanh makes the order irrelevant to the output
    order = np.argsort(-np.abs(v_w))
    W_s = W_e[:, order]
    v_s = v_w[order]
    c_s = c[:, order]

    corr = None
    use_corr = CORR and kf == 0
    if use_corr:
        # see CORR comment at top: fold ehat@(dW u) into the top-|v| weight
        # columns, and precompute the de@(W u_b) map for DVE injection
        W8q = (W_s * np.float32(W8_SCALE)).astype(E4)
        dW = W_s - W8q.astype(np.float32) / np.float32(W8_SCALE)
        sigP = np.sqrt((W_s * W_s).sum(axis=0))      # [Hd]
        mf = _gh_mean_tanhp(0.0, sigP)               # [Hd] col-only m
        u_f = (mf * v_s).astype(np.float32)
        f = dW @ u_f                                 # [He]
        hstar = np.arange(CORR_FOLD_K)
        beta = 1.0 / (CORR_FOLD_K * v_s[hstar] * mf[hstar])
        W_s = W_s.copy()
        W_s[:, hstar] += np.outer(f, beta)
        mA = _gh_mean_tanhp(c_s, sigP[None, :])      # [B, Hd]
        # wu uses the unfolded sorted W (matches the error sim exactly)
        wu = np.ascontiguousarray(W_e[:, order]) @ (mA * v_s[None, :]).T

    whb = wh8 = None
    if kf > 0:
        # whb[p, (hd*KT+k)*128+m] = W_s[k*128+p, hd*128+m]  (hd-major so
        # group 0's weights are one contiguous leading chunk -- the first
        # block's matmuls start after 1/kf of the weight DMA)
        whb = np.ascontiguousarray(
            W_s[:, :kf * 128].reshape(KT, 128, kf, 128)
            .transpose(1, 2, 0, 3).reshape(128, -1).astype(BF))
    if kf < HT:
        W8 = np.asarray(W_s[:, kf * 128:] * np.float32(W8_SCALE), dtype=E4)
        # wh8[p, k2*HF+hf, j, m] = W8[k2*256 + j*128 + p, hf*128+m]
        wh8 = np.ascontiguousarray(
            W8.reshape(KT2, 2, 128, HF, 128).transpose(2, 0, 3, 1, 4)
            .reshape(128, KT2 * HF, 2, 128))
        if sw:
            # DoubleRowSwInterleave byte order (probed on HW):
            # flat[p, t, 2*(127-m)+j] = wdr[p, t, j, m]
            wh8 = np.ascontiguousarray(
                wh8[:, :, :, ::-1].transpose(0, 1, 3, 2)
                .reshape(128, KT2 * HF, 2, 128))
    vw = np.ascontiguousarray(v_s.reshape(HT, 128).T.astype(np.float32))

    in_maps = []
    for ci in range(N_CORES):
        bsl = slice(ci * BPC, (ci + 1) * BPC)
        m = {"cb": np.ascontiguousarray(
                 c_s[bsl].reshape(BPC, HT, 128).transpose(2, 0, 1)
                 .reshape(128, -1)),
             "vw": vw,
             "ones": np.ones((128, 1),
                             np.float16 if ACC_F16 else np.float32)}
        if kf > 0:
            m["whb"] = whb
            m["encTb"] = np.ascontiguousarray(
                enc[bsl].transpose(0, 2, 1).astype(BF))      # [BPC, He, S]
        if kf < HT:
            m["wh8"] = wh8
            E16 = np.asarray(enc[bsl] * np.float32(E8_SCALE), dtype=E4)
            # encT8[b, k2, p, j, s] = E16[b, s, k2*256 + j*128 + p]
            m["encT8"] = np.ascontiguousarray(
                E16.reshape(BPC, S, KT2, 2, 128).transpose(0, 2, 4, 3, 1))
            if use_corr:
                de = enc[bsl] - E16.astype(np.float32) / np.float32(E8_SCALE)
                cc = np.empty((BPC, S), np.float32)
                for i, b in enumerate(range(bsl.start, bsl.stop)):
                    cc[i] = de[i] @ wu[:, b]
                m["corr"] = np.ascontiguousarray(cc.reshape(1, BPC * S))
        in_maps.append(m)
    return in_maps


def _np_mm_dt():
    if MM_DTYPE == "bf16":
        import ml_dtypes
        return ml_dtypes.bfloat16
    if MM_DTYPE == "fp8dr":
        import ml_dtypes
        return ml_dtypes.float8_e4m3
    return np.float32


def prepare_in_maps(hidden, encoder_outputs, W_attn, b_attn, v_w,
                    hyb=False):
    if DEFAULT_MODE.startswith("vmix"):
        kf, sw, _pair = _parse_vmix(DEFAULT_MODE)
        return prepare_in_maps_vmix(hidden, encoder_outputs, W_attn, b_attn,
                                    v_w, kf=kf, sw=sw)
    mmdt = _np_mm_dt()
    hidden = np.ascontiguousarray(np.asarray(hidden, dtype=np.float32))
    enc = np.asarray(encoder_outputs, dtype=np.float32)
    W_attn = np.asarray(W_attn, dtype=np.float32)
    b_attn = np.asarray(b_attn, dtype=np.float32)
    v_w = np.asarray(v_w, dtype=np.float32)

    h = hidden[-1]                      # [B, He]
    W_h = W_attn[:He]                   # [He, Hd]
    W_e = W_attn[He:]                   # [He, Hd]
    c = (h @ W_h + b_attn).astype(np.float32)   # [B, Hd]

    wh8 = None
    if hyb:
        import ml_dtypes
        f8 = ml_dtypes.float8_e4m3
        # fp8 part: He[0:256]; wh8[p, hd, s, m] = W_e[s*128+p, hd*128+m]
        wh8 = np.ascontiguousarray(
            W_e[:256].reshape(2, 128, HT, 128).transpose(1, 2, 0, 3)
            .reshape(128, HT, 2, 128).astype(f8))
        wh = np.ascontiguousarray(
            W_e[256:].reshape(KT - 2, 128, HT, 128).transpose(1, 0, 2, 3)
            .reshape(128, -1).astype(np.float32))
    elif MM_DTYPE == "fp8dr":
        # wh[p, k2*HT+hd, s, m] = W_e[k2*256 + s*128 + p, hd*128+m]
        wh = np.ascontiguousarray(
            W_e.reshape(KT2, 2, 128, HT, 128).transpose(2, 0, 3, 1, 4)
            .reshape(128, KT2 * HT, 2, 128).astype(mmdt))
    else:
        # wh[p, (k*HT+hd)*128+m] = W_e[k*128+p, hd*128+m]
        wh = np.ascontiguousarray(
            W_e.reshape(KT, 128, HT, 128).transpose(1, 0, 2, 3).reshape(128, -1)
            .astype(mmdt))
    # vw[p, hd] = v_w[hd*128+p]
    vw_dt = np.float32 if (VW_F32 or MM_DTYPE == "fp8dr") else mmdt
    vw = np.ascontiguousarray(v_w.reshape(HT, 128).T.astype(vw_dt))

    in_maps = []
    for ci in range(N_CORES):
        bsl = slice(ci * BPC, (ci + 1) * BPC)
        encT8 = None
        if hyb:
            import ml_dtypes
            f8 = ml_dtypes.float8_e4m3
            # encT8[b, p, s, n] = enc[b, n, s*128 + p] for He[0:256]
            encT8 = np.ascontiguousarray(
                enc[bsl, :, :256].reshape(BPC, S, 2, 128)
                .transpose(0, 3, 2, 1).astype(f8))
            encT = np.ascontiguousarray(
                enc[bsl, :, 256:].transpose(0, 2, 1).astype(np.float32))
        elif MM_DTYPE == "fp8dr":
            # encT[b, k2, p, s, n] = enc[b, n, k2*256 + s*128 + p]
            encT = np.ascontiguousarray(
                enc[bsl].reshape(BPC, S, KT2, 2, 128)
                .transpose(0, 2, 4, 3, 1).astype(mmdt))
        else:
            encT = np.ascontiguousarray(
                enc[bsl].transpose(0, 2, 1).astype(mmdt))  # [BPC, He, S]
        cb = np.ascontiguousarray(
            c[bsl].reshape(BPC, HT, 128).transpose(2, 0, 1).reshape(128, -1))
        m = {"encT": encT, "wh": wh, "cb": cb, "vw": vw,
             "ones": np.ones((128, 1), np.float32)}
        if hyb:
            m["encT8"] = encT8
            m["wh8"] = wh8
        in_maps.append(m)
    return in_maps


def kernel(hidden, encoder_outputs, W_attn, b_attn, v_w):
    nc = build_nc()
    if DEFAULT_MODE.startswith("vmix"):
        kf, sw, _pair = _parse_vmix(DEFAULT_MODE)
        in_maps = prepare_in_maps_vmix(hidden, encoder_outputs, W_attn,
                                       b_attn, v_w, kf=kf, sw=sw)
    else:
        in_maps = prepare_in_maps(hidden, encoder_outputs, W_attn, b_attn, v_w)
    res = run_bass_kernel_spmd(nc, in_maps, core_ids=list(range(N_CORES)))
    full = np.concatenate([res.results[i]["out"] for i in range(N_CORES)],
                          axis=0)
    if (DEFAULT_MODE.startswith("vmix") and HOST_SOFTMAX and PAIR2
            and _parse_vmix(DEFAULT_MODE)[0] == 0):
        # device shipped raw scores; finalize the softmax here
        e = np.exp(full - full.max(axis=1, keepdims=True))
        full = (e / e.sum(axis=1, keepdims=True)).astype(np.float32)
    return full

